# revision 17
# baseline (speedup 1.0000x reference)
"""LocalAutoCorr2D Trainium2 kernel.

out[b,c,i,j,dy,dx] = sum_{y,x valid} x[b,c,4i+y,4j+x] * x[b,c,4i+y+sy,4j+x+sx]
with (sy,sx) = (dy-4, dx-4), windows 8x8 at stride 4 on a 96x96 image,
zero-padded at window boundaries.

Strategy (per core, batch-sharded over 8 cores):
  - out[s] == out[-s] (autocorr symmetry) -> only 40 canonical shift classes.
  - x is host-prepped into PHASE-MAJOR fp16 layouts with the image split
    as u = 4g + r (r the phase, innermost axis c), so every matmul rhs
    view (fixed r, a 23-g window, all c) is one FLAT contiguous slice:
    the PE streams at full rate (a strided/multi-dim rhs runs ~2x slower).
    The 5 shift copies along the partition axis are host-stacked in the
    free dim, so DVE products never need cross-partition operands.
  - Per shift, the box-sum runs in the orientation that contracts the
    LONGER box extent through the 0/1 A-matrix matmul (partition axis)
    and accumulates the shorter extent via PSUM across passes:
    passes = min(8-|sy|, 8-|sx|). Two host layouts: xa ([h, ...] stack
    over sy) for horizontal-pass shifts, xw ([w, ...] stack over sx) for
    vertical-pass shifts. The A-matrix pattern is identical either way.
  - Products on the Vector engine (fp16 2x mode, flat contiguous views,
    all offsets multiples of C=64 so alignment is automatic); the (0,0)
    square runs on the Act engine. Warmup matmuls during the input DMA
    keep the PE p-state ramped. Scalar evacuates PSUM -> SBUF (fp16);
    GpSimd queues the output DMAs.
"""

import functools
import os
import sys

import numpy as np

sys.path.insert(0, "/opt/trn_rl_repo")

import concourse.bass as bass  # noqa: E402
import concourse.bacc as bacc  # noqa: E402
import concourse.mybir as mybir  # noqa: E402
from concourse import bass_utils  # noqa: E402
from concourse.tile import TileContext  # noqa: E402

B, C, H, W = 8, 64, 96, 96
KH = KW = 8
SH = SW = 4
NH = NW = 23
NCORES = 8

JP = 24           # g positions per r-block (u = 4g + r)
BLK = C * JP      # 1536 elements per r-block
FLAT = 4 * BLK    # 6144
NV = 5            # partition-shift copies v=0..4 stacked in the free dim
NVW = 4           # xw only needs v=0..3 (vertical-pass shifts have |sx|<=3)
BASE = 64         # leading pad elements (AP validity for negative offsets)
TAIL = 128
XCOLS = BASE + NV * FLAT + TAIL
XCOLSW = BASE + NVW * FLAT + TAIL
N_CHUNKS = [(0, 512), (512, 1024), (1024, 1472)]  # flat cols per PSUM bank
N_WARM = 52       # PE warmup: must bridge until the first product is ready

fp32 = mybir.dt.float32
fp16 = mybir.dt.float16


def _canonical_cells():
    """Map canonical shift (sy>=0, sx) -> list of output cells (dy,dx)."""
    cells = {}
    for dy in range(8):
        for dx in range(8):
            sy, sx = dy - 4, dx - 4
            key = (sy, sx) if (sy > 0 or (sy == 0 and sx >= 0)) else (-sy, -sx)
            cells.setdefault(key, []).append((dy, dx))
    assert len(cells) == 40
    return cells


def _is_w(key):
    """Vertical-pass (w-contracting) orientation when the x-extent of the
    box is shorter than the y-extent: passes = min of the two."""
    sy, sx = key
    return sy > abs(sx)


def _w_rep(key):
    """Representative (syw, sxw) with sxw >= 0 for the xw stack."""
    sy, sx = key
    return (sy, sx) if sx >= 0 else (-sy, -sx)


def _amat_np():
    """Box-sum matrices, stacked: A[u, p*23+g] = 1 if 0 <= u-4g < 8-p.
    Used as the vertical matrix (u=h, p=sy) and, identically, as the
    horizontal matrix (u=w, p=sxw)."""
    a = np.zeros((H, 5 * NH), np.float16)
    for p in range(5):
        for g in range(NH):
            a[4 * g : 4 * g + 8 - p, p * NH + g] = 1.0
    return a


def _stack(img, nv):
    """[U, V, C] fp32 (partition axis U first) -> phase-major fp16 stack
    [U, BASE + nv*FLAT + TAIL]: col v*FLAT + (r, g, c) = img[u+v, 4g+r, c]."""
    U = img.shape[0]
    pm = img.reshape(U, JP, 4, C).transpose(0, 2, 1, 3)  # [u, r, g, c]
    flat = np.ascontiguousarray(pm.reshape(U, FLAT)).astype(np.float16)
    out = np.zeros((U, BASE + nv * FLAT + TAIL), np.float16)
    for v in range(nv):
        out[0 : U - v, BASE + v * FLAT : BASE + (v + 1) * FLAT] = flat[v:U]
    return out


def _prep_x(xb):
    """[C,H,W] fp32 -> (xa [H, XCOLS], xw [W, XCOLSW]) fp16 stacks."""
    xa = _stack(xb.transpose(1, 2, 0), NV)        # [h, w, c] stack over sy
    xw = _stack(xb.transpose(2, 1, 0), NVW)       # [w, h, c] stack over sx
    return xa, xw


def _order(cells):
    """Grouped to match the input DMA chunk arrival order, so the DVE
    never stalls on a stack block that hasn't landed yet:
    xa-v0 | xw-v0 | xa-v1 | xa-v2 | xw-v1 | xa-v3 | xw-v2 | xa-v4 | xw-v3
    """
    groups = [
        [(0, 0), (0, 1), (0, 2), (0, 3), (0, 4)],          # xa-v0
        [(1, 0), (2, 0), (3, 0), (4, 0)],                  # xw-v0
        [(1, 1), (1, -1), (1, 2), (1, -2), (1, 3), (1, -3),
         (1, 4), (1, -4)],                                 # xa-v1
        [(2, 4), (2, -4), (2, 2), (2, -2), (2, 3), (2, -3)],  # xa-v2
        [(2, 1), (2, -1), (3, 1), (3, -1), (4, 1), (4, -1)],  # xw-v1
        [(3, 3), (3, -3), (3, 4), (3, -4)],                # xa-v3
        [(3, 2), (3, -2), (4, 2), (4, -2)],                # xw-v2
        [(4, 4)],                                          # xa-v4
        [(4, 3), (4, -3)],                                 # xw-v3
    ]
    o = [k for g in groups for k in g]
    assert sorted(o) == sorted(cells.keys())
    return o


def build_nc():
    nc = bacc.Bacc()
    xa_dram = nc.dram_tensor("xa", [H, XCOLS], fp16, kind="ExternalInput")
    xw_dram = nc.dram_tensor("xw", [W, XCOLSW], fp16, kind="ExternalInput")
    amat_dram = nc.dram_tensor("amat", [H, 5 * NH], fp16, kind="ExternalInput")
    out_dram = nc.dram_tensor("out", [8, 8, NH, NW * C], fp16,
                              kind="ExternalOutput")

    cells = _canonical_cells()
    order = _order(cells)

    with TileContext(nc) as tc:
        with (
            tc.tile_pool(name="const", bufs=1) as cpool,
            tc.tile_pool(name="q", bufs=4) as qpool,
            tc.tile_pool(name="o", bufs=3) as opool,
            tc.tile_pool(name="ps", bufs=2, space="PSUM") as ppool,
            tc.tile_pool(name="pw", bufs=1, space="PSUM") as wpool,
        ):
            amat_t = cpool.tile([H, 5 * NH], fp16)
            nc.sync.dma_start(amat_t, amat_dram[:, :])
            xa_t = cpool.tile([H, XCOLS], fp16)
            xw_t = cpool.tile([W, XCOLSW], fp16)
            # lowest v first so early consumers unblock first; xw v=0
            # right after xa v=0 (the (1,0) W-shift is the 6th consumer);
            # xa v=0 in halves so the (0,0) square starts on the first
            ca = [(xa_t, xa_dram, lo, hi) for lo, hi in zip(
                [0, BASE + FLAT // 2] +
                [BASE + v * FLAT for v in range(1, NV)] + [XCOLS][:1],
                [BASE + FLAT // 2] +
                [BASE + v * FLAT for v in range(1, NV)] + [XCOLS])]
            cw = [(xw_t, xw_dram, lo, hi) for lo, hi in zip(
                [0] + [BASE + v * FLAT for v in range(1, NVW)],
                [BASE + v * FLAT for v in range(1, NVW)] + [XCOLSW])]
            dmas = [ca[0], ca[1], cw[0], ca[2], ca[3], cw[1], ca[4],
                    cw[2], ca[5], cw[3]]
            for k, (t, dram, lo, hi) in enumerate(dmas):
                eng = nc.gpsimd if k % 2 == 0 else nc.sync
                eng.dma_start(t[:, lo:hi], dram[:, lo:hi])

            # PE warmup: keep the p-state ramped while inputs stream in
            # (memset on the otherwise-idle DVE so warmup starts at once)
            wt = cpool.tile([H, 512], fp16)
            nc.vector.memset(wt, 0.0)
            warm_pt = wpool.tile([NH, 512], fp32)
            for _ in range(N_WARM):
                nc.tensor.matmul(warm_pt, wt[:, 0:NH], wt,
                                 start=True, stop=True)

            for key in order:
                sy, sx = key
                if _is_w(key):
                    syw, sxw = _w_rep(key)
                    p_shift, f_shift = sxw, syw
                    stack_t = xw_t
                    passes = list(range(max(0, -syw), 8 - max(0, syw)))
                else:
                    p_shift, f_shift = sy, sx
                    stack_t = xa_t
                    passes = list(range(max(0, -sx), 8 - max(0, sx)))
                s = f_shift % 4          # python %: s in [0,4) for negatives
                a = (f_shift - s) // 4
                pv = H - p_shift
                q = qpool.tile([H, FLAT], fp16, tag="q")

                def mul(flo, fhi, delta):
                    nc.vector.tensor_mul(
                        q[0:pv, flo:fhi],
                        stack_t[0:pv, BASE + flo : BASE + fhi],
                        stack_t[0:pv, BASE + delta + flo : BASE + delta + fhi],
                    )

                if key == (0, 0):
                    # x^2 on the Act engine: frees the DVE and starts as
                    # soon as each half of the v=0 DMA chunk lands
                    for lo, hi in [(0, FLAT // 2), (FLAT // 2, FLAT)]:
                        nc.scalar.activation(
                            q[:, lo:hi], xa_t[:, BASE + lo : BASE + hi],
                            mybir.ActivationFunctionType.Square,
                        )
                else:
                    lenA = (4 - s) * BLK
                    mul(0, lenA, p_shift * FLAT + s * BLK + C * a)
                    if s:
                        mul(lenA, FLAT,
                            p_shift * FLAT + (s - 4) * BLK + C * (a + 1))

                a_k = amat_t[0:pv, p_shift * NH : (p_shift + 1) * NH]
                o_t = opool.tile([NH, NW * C], fp16, tag="o")
                for ci, (n0, n1) in enumerate(N_CHUNKS):
                    pt = ppool.tile([NH, n1 - n0], fp32, tag=f"ps{ci}")
                    for pi, y in enumerate(passes):
                        base = (y % 4) * BLK + C * (y // 4)
                        rhs = q[0:pv, base + n0 : base + n1]
                        nc.tensor.matmul(
                            pt, a_k, rhs,
                            start=(pi == 0), stop=(pi == len(passes) - 1),
                        )
                    nc.scalar.copy(o_t[:, n0:n1], pt)
                for (dy, dx) in cells[key]:
                    nc.gpsimd.dma_start(out_dram[dy, dx], o_t)

    if not nc.is_finalized():
        nc.finalize()
    return nc


@functools.lru_cache(maxsize=1)
def _get_nc():
    return build_nc()


def _in_maps(x):
    amat = _amat_np()
    maps = []
    for b in range(NCORES):
        xa, xw = _prep_x(x[b])
        maps.append({"xa": xa, "xw": xw, "amat": amat})
    return maps


def _w_cells():
    cells = _canonical_cells()
    out = set()
    for key, cs in cells.items():
        if _is_w(key):
            out.update(cs)
    return out


def kernel(**inputs) -> np.ndarray:
    x = np.asarray(inputs["x"], dtype=np.float32)
    assert x.shape == (B, C, H, W)
    nc = _get_nc()
    in_maps = _in_maps(x)
    res = bass_utils.run_bass_kernel_spmd(
        nc, in_maps, core_ids=list(range(NCORES)),
        trace=bool(int(os.environ.get("KERNEL_TRACE", "0"))),
    )
    outs = np.stack([r["out"] for r in res.results])  # [B, dy, dx, ?, ?]
    outs = outs.reshape(B, 8, 8, NH, NW, C).astype(np.float32)
    # w-oriented cells come out [j, i, c]: swap back to [i, j, c]
    wc = _w_cells()
    full = outs.copy()
    for (dy, dx) in wc:
        full[:, dy, dx] = outs[:, dy, dx].transpose(0, 2, 1, 3)
    # [B, dy, dx, i, j, c] -> [B, c, i, j, dy, dx]
    full = full.transpose(0, 5, 3, 4, 1, 2)
    return np.ascontiguousarray(full).astype(np.float32)


if __name__ == "__main__":
    rng = np.random.default_rng(0)
    x = rng.standard_normal((B, C, H, W), dtype=np.float32)
    y = kernel(x=x)
    print("out", y.shape, y.dtype, float(np.abs(y).max()))


# revision 25
# speedup vs baseline: 1.1051x; 1.1051x over previous
"""LocalAutoCorr2D Trainium2 kernel.

out[b,c,i,j,dy,dx] = sum_{y,x valid} x[b,c,4i+y,4j+x] * x[b,c,4i+y+sy,4j+x+sx]
with (sy,sx) = (dy-4, dx-4), windows 8x8 at stride 4 on a 96x96 image,
zero-padded at window boundaries.

Strategy (per core, batch-sharded over 8 cores):
  - out[s] == out[-s] (autocorr symmetry) -> only 40 canonical shift classes.
  - x is host-prepped into a PHASE-MAJOR fp16 layout [h, (r, j, c)] with
    w = 4j + r and c innermost, so every matmul rhs view (fixed r, a
    23-j window, all c) is one FLAT contiguous slice: the PE streams at
    full rate (a strided or multi-dim rhs runs at ~half rate). The 5
    vertical shifts v=0..4 are also host-stacked along the free dim, so
    DVE products never need cross-partition operands.
  - Per shift: product Q = x .* shift(x) on the Vector engine (fp16 2x
    mode, flat contiguous views; all shift offsets are multiples of C=64
    elements, so alignment is automatic). Vertical box-sum via 0/1-weight
    matmul (h on partitions), horizontal box-sum folded into PSUM
    accumulation across <=8 matmuls over flat rhs slices of Q.
  - Warmup matmuls bridge the PE from program start until the first
    product is ready: a PE idle gap after ramping can drop the engine
    into a sticky half-clock state for a long stretch.
  - The (0,0) square runs on the Act engine (frees the DVE, starts as
    the first input DMA chunk lands). Scalar evacuates PSUM -> SBUF as
    fp16; each canonical shift is written to DRAM once and the host
    mirrors the 24 symmetric duplicate cells.
"""

import functools
import os
import sys

import numpy as np

sys.path.insert(0, "/opt/trn_rl_repo")

import concourse.bass as bass  # noqa: E402
import concourse.bacc as bacc  # noqa: E402
import concourse.mybir as mybir  # noqa: E402
from concourse import bass_utils  # noqa: E402
from concourse.tile import TileContext  # noqa: E402

B, C, H, W = 8, 64, 96, 96
KH = KW = 8
SH = SW = 4
NH = NW = 23
NCORES = 8

JP = 24           # j' positions per r-block (w = 4j + r)
BLK = C * JP      # 1536 elements per r-block
FLAT = 4 * BLK    # 6144
NV = 5            # vertical shift copies v=0..4 stacked in the free dim
BASE = 64         # leading pad elements (AP validity for negative offsets)
TAIL = 128
XCOLS = BASE + NV * FLAT + TAIL
N_CHUNKS = [(0, 512), (512, 1024), (1024, 1472)]  # flat cols per PSUM bank
N_WARM = 42       # PE warmup: must bridge until the first product is ready

fp32 = mybir.dt.float32
fp16 = mybir.dt.float16


def _canonical_cells():
    """Map canonical shift (sy>=0, sx) -> list of output cells (dy,dx)."""
    cells = {}
    for dy in range(8):
        for dx in range(8):
            sy, sx = dy - 4, dx - 4
            key = (sy, sx) if (sy > 0 or (sy == 0 and sx >= 0)) else (-sy, -sx)
            cells.setdefault(key, []).append((dy, dx))
    assert len(cells) == 40
    return cells


def _amat_np():
    """Vertical box-sum matrices, stacked: A[h, sy*23+i] = 1 if 0<=h-4i<8-sy,
    plus a trailing 23x23 identity block (stage-2 of the T4 reduction)."""
    a = np.zeros((H, 6 * NH), np.float16)
    for sy in range(5):
        for i in range(NH):
            a[4 * i : 4 * i + 8 - sy, sy * NH + i] = 1.0
    a[np.arange(NH), 5 * NH + np.arange(NH)] = 1.0
    return a


def _prep_x(xb):
    """[C,H,W] fp32 -> xa phase-major fp16 [H, XCOLS].

    xa[h, BASE + v*FLAT + (r,j,c)] = x[h+v, c, 4j+r]  (0 beyond the image)."""
    t = xb.transpose(1, 2, 0)  # [h, w, c]
    pm = t.reshape(H, JP, 4, C).transpose(0, 2, 1, 3)  # [h, r, j, c]
    flat = np.ascontiguousarray(pm.reshape(H, FLAT)).astype(np.float16)
    xa = np.zeros((H, XCOLS), np.float16)
    for v in range(NV):
        xa[0 : H - v, BASE + v * FLAT : BASE + (v + 1) * FLAT] = flat[v:H]
    return xa


def _order(cells):
    """sy=0 shifts first (their stack block lands first), then by growing
    |sx| so the PE builds backlog early; (4,0) moved to the very end so
    the PE drains on a big-Lx shift instead of starving."""
    o = sorted(cells.keys(), key=lambda s: (s[0], abs(s[1])))
    o.remove((4, 0))
    o.append((4, 0))
    return o


def build_nc():
    nc = bacc.Bacc()
    xa_dram = nc.dram_tensor("xa", [H, XCOLS], fp16, kind="ExternalInput")
    amat_dram = nc.dram_tensor("amat", [H, 6 * NH], fp16, kind="ExternalInput")
    out_dram = nc.dram_tensor("out", [40, NH, NW * C], fp16,
                              kind="ExternalOutput")

    cells = _canonical_cells()
    order = _order(cells)

    with TileContext(nc) as tc:
        with (
            tc.tile_pool(name="const", bufs=1) as cpool,
            tc.tile_pool(name="q", bufs=4) as qpool,
            tc.tile_pool(name="o", bufs=3) as opool,
            tc.tile_pool(name="t4", bufs=2) as t4pool,
            tc.tile_pool(name="ps", bufs=2, space="PSUM") as ppool,
            tc.tile_pool(name="pw", bufs=1, space="PSUM") as wpool,
        ):
            amat_t = cpool.tile([H, 6 * NH], fp16)
            nc.sync.dma_start(amat_t, amat_dram[:, :])
            xa_t = cpool.tile([H, XCOLS], fp16)
            # PE warmup: keep the p-state ramped while inputs stream in
            wt = cpool.tile([H, 512], fp16)
            nc.vector.memset(wt, 0.0)
            warm_pt = wpool.tile([NH, 512], fp32)
            for _ in range(N_WARM):
                nc.tensor.matmul(warm_pt, wt[:, 0:NH], wt,
                                 start=True, stop=True)
            # chunked so the v=0 block (first consumer) lands first; v=0
            # in halves so the (0,0) square can start on the first half.
            # Issued from different engines: each engine's DGE feeds its
            # own DMA queue, so the chunks transfer in parallel instead
            # of serializing on one queue.
            bounds = [0, BASE + FLAT // 2] + \
                [BASE + v * FLAT for v in range(1, NV)] + [XCOLS]
            # v=0 halves go to two different queues so they land in
            # parallel: the (0,0) square (and everything after) starts
            # a few us sooner
            issuers = [nc.gpsimd, nc.sync, nc.gpsimd, nc.sync,
                       nc.gpsimd, nc.sync]
            for eng, (lo, hi) in zip(issuers, zip(bounds[:-1], bounds[1:])):
                eng.dma_start(xa_t[:, lo:hi], xa_dram[:, lo:hi])

            def emit_product(sy, sx, q, hv):
                s = sx % 4          # python %: s in [0,4) also for sx<0
                a = (sx - s) // 4

                def mul(flo, fhi, delta):
                    # q[h, f] = x[h, f] * x[h+sy, f+delta-sy*FLAT] on
                    # f in [flo, fhi); the sy shift is baked into the stack.
                    off = BASE + delta
                    nc.vector.tensor_mul(
                        q[0:hv, flo:fhi],
                        xa_t[0:hv, BASE + flo : BASE + fhi],
                        xa_t[0:hv, off + flo : off + fhi],
                    )

                if (sy, sx) == (0, 0):
                    # x^2 split across Act and DVE quarters so q(0,0) is
                    # ready ~as soon as the v=0 chunks land
                    qr = FLAT // 4
                    for k in range(4):
                        lo, hi = k * qr, (k + 1) * qr
                        if k % 2 == 0:
                            nc.scalar.activation(
                                q[:, lo:hi], xa_t[:, BASE + lo : BASE + hi],
                                mybir.ActivationFunctionType.Square,
                            )
                        else:
                            nc.vector.tensor_mul(
                                q[:, lo:hi],
                                xa_t[:, BASE + lo : BASE + hi],
                                xa_t[:, BASE + lo : BASE + hi],
                            )
                else:
                    lenA = (4 - s) * BLK
                    mul(0, lenA, sy * FLAT + s * BLK + C * a)
                    if s:
                        mul(lenA, FLAT,
                            sy * FLAT + (s - 4) * BLK + C * (a + 1))

            def emit_direct(sy, sx):
                hv = H - sy
                q = qpool.tile([H, FLAT], fp16, tag="q", name="qd")
                emit_product(sy, sx, q, hv)
                a_k = amat_t[0:hv, sy * NH : (sy + 1) * NH]
                xlist = list(range(max(0, -sx), 8 - max(0, sx)))
                o_t = opool.tile([NH, NW * C], fp16, tag="o", name="od")
                for ci, (n0, n1) in enumerate(N_CHUNKS):
                    pt = ppool.tile([NH, n1 - n0], fp32, tag=f"ps{ci}",
                                    name="ptd")
                    for xi, xx in enumerate(xlist):
                        base = (xx % 4) * BLK + C * (xx // 4)
                        rhs = q[0:hv, base + n0 : base + n1]
                        nc.tensor.matmul(
                            pt, a_k, rhs,
                            start=(xi == 0), stop=(xi == len(xlist) - 1),
                        )
                    nc.scalar.copy(o_t[:, n0:n1], pt)
                    # chunk DMA'd as soon as it's evacuated: the output
                    # queue drains continuously instead of piling up at
                    # the end. One write per canonical shift; the host
                    # mirrors the symmetric duplicates (out[s] == out[-s])
                    nc.gpsimd.dma_start(
                        out_dram[order.index((sy, sx))][:, n0:n1],
                        o_t[:, n0:n1])

            def emit_t4_stage1(sy):
                # width-4 phase sums for an Lx=8 (sx=0) shift: T4[i, j, c]
                # = sum_r sum_h A[h,i] Q[h, (r,j,c)], j in [0, 24)
                hv = H - sy
                q = qpool.tile([H, FLAT], fp16, tag="q", name="qt")
                emit_product(sy, 0, q, hv)
                a_k = amat_t[0:hv, sy * NH : (sy + 1) * NH]
                t4sb = t4pool.tile([NH, JP * C], fp16, tag="t4")
                for ci, n0 in enumerate((0, 512, 1024)):
                    pt = ppool.tile([NH, 512], fp32, tag=f"ps{ci}",
                                    name="ptt")
                    for ri in range(4):
                        rhs = q[0:hv, ri * BLK + n0 : ri * BLK + n0 + 512]
                        nc.tensor.matmul(pt, a_k, rhs,
                                         start=(ri == 0), stop=(ri == 3))
                    nc.scalar.copy(t4sb[:, n0 : n0 + 512], pt)
                return t4sb

            def emit_t4_finish(sy, t4sb):
                # out[i, j, c] = T4[i, j, c] + T4[i, j+1, c] via two
                # identity-matmul passes over the evacuated T4
                ident = amat_t[0:NH, 5 * NH : 6 * NH]
                o_t = opool.tile([NH, NW * C], fp16, tag="o", name="ot")
                for ci, (n0, n1) in enumerate(N_CHUNKS):
                    pt = ppool.tile([NH, n1 - n0], fp32, tag=f"ps{ci}",
                                    name="ptf")
                    nc.tensor.matmul(pt, ident, t4sb[0:NH, n0:n1],
                                     start=True, stop=False)
                    nc.tensor.matmul(pt, ident, t4sb[0:NH, C + n0 : C + n1],
                                     start=False, stop=True)
                    nc.scalar.copy(o_t[:, n0:n1], pt)
                    nc.gpsimd.dma_start(
                        out_dram[order.index((sy, 0))][:, n0:n1],
                        o_t[:, n0:n1])

            pending = None
            for (sy, sx) in order:
                if sx == 0:
                    if pending is not None:
                        emit_t4_finish(*pending)
                    pending = (sy, emit_t4_stage1(sy))
                else:
                    emit_direct(sy, sx)
                    if pending is not None:
                        emit_t4_finish(*pending)
                        pending = None
            if pending is not None:
                emit_t4_finish(*pending)

    if not nc.is_finalized():
        nc.finalize()
    return nc


@functools.lru_cache(maxsize=1)
def _get_nc():
    return build_nc()


def _in_maps(x):
    amat = _amat_np()
    return [{"xa": _prep_x(x[b]), "amat": amat} for b in range(NCORES)]


def kernel(**inputs) -> np.ndarray:
    x = np.asarray(inputs["x"], dtype=np.float32)
    assert x.shape == (B, C, H, W)
    nc = _get_nc()
    in_maps = _in_maps(x)
    res = bass_utils.run_bass_kernel_spmd(
        nc, in_maps, core_ids=list(range(NCORES)),
        trace=bool(int(os.environ.get("KERNEL_TRACE", "0"))),
    )
    outs = np.stack([r["out"] for r in res.results])  # [B, 40, i, (j c)]
    outs = outs.reshape(B, 40, NH, NW, C).astype(np.float32)
    cells = _canonical_cells()
    order = _order(cells)
    full = np.empty((B, 8, 8, NH, NW, C), np.float32)
    for ki, key in enumerate(order):
        for (dy, dx) in cells[key]:
            full[:, dy, dx] = outs[:, ki]
    # [B, dy, dx, i, j, c] -> [B, c, i, j, dy, dx]
    full = full.transpose(0, 5, 3, 4, 1, 2)
    return np.ascontiguousarray(full).astype(np.float32)


if __name__ == "__main__":
    rng = np.random.default_rng(0)
    x = rng.standard_normal((B, C, H, W), dtype=np.float32)
    y = kernel(x=x)
    print("out", y.shape, y.dtype, float(np.abs(y).max()))
